# revision 6
# baseline (speedup 1.0000x reference)
"""Data-parallel 8-core Trainium2 kernel for nn_ACORPolicy_6897717478056.

Strategy (per sharding_hint): data-parallel over the batch dim — each of the
8 NeuronCores gets B/8 = 4 episodes.  All top-k / gather / consensus work is
batch-local, so the forward needs no cross-core collectives; the full outputs
are re-assembled on the host by concatenating the per-core shards.

Self-contained: all shapes/dims hardcoded; no sibling imports.
"""

import numpy as np
import jax
import jax.numpy as jnp

try:  # persistent compile cache so repeat invocations skip neuronx-cc
    jax.config.update("jax_compilation_cache_dir", "/tmp/jax_cache")
    jax.config.update("jax_persistent_cache_min_compile_time_secs", 0.5)
except Exception:
    pass

# ---- dims (hardcoded from the problem spec) ----
B, A, OBS_DIM, POS_DIM, HIST_W = 32, 256, 64, 2, 8
K_NEIGH, LEADER_K = 16, 8
INTRA, INTER = 2, 2
EPS = 1e-8
N_CORES = 8
B_LOC = B // N_CORES  # 4 episodes per core


def _ln(x, g, b):
    mu = x.mean(-1, keepdims=True)
    v = ((x - mu) ** 2).mean(-1, keepdims=True)
    return (x - mu) / jnp.sqrt(v + 1e-5) * g + b


def _mlp_apply(layers, x):
    for blk in layers[:-1]:
        x = jax.nn.gelu(_ln(x @ blk["W"] + blk["b"], blk["g"], blk["beta"]),
                        approximate=False)
    last = layers[-1]
    return x @ last["W"] + last["b"]


def _gather(x, idx):
    return jax.vmap(lambda xb, ib: xb[ib])(x, idx)


def _forward_shard(obs, positions, history, params):
    """Identical math to the reference forward, on a [B_LOC, A, ...] shard."""
    Bs = obs.shape[0]
    beh = _mlp_apply(params["beh_enc"], history.mean(axis=2))
    oe = _mlp_apply(params["obs_enc"], obs)
    OBS_EMB = oe.shape[-1]

    diff = positions[:, :, None, :] - positions[:, None, :, :]
    dist_raw = jnp.sqrt(jnp.maximum((diff * diff).sum(-1), 1e-12))
    eye = jnp.eye(A, dtype=bool)
    dist = jnp.where(eye, jnp.inf, dist_raw)
    neg, neighbor_idx = jax.lax.top_k(-dist, K_NEIGH)
    membership = jax.nn.softmax(neg, axis=-1)

    beh_nb = _gather(beh, neighbor_idx)
    trust_in = jnp.concatenate(
        [jnp.broadcast_to(oe[:, :, None, :], (Bs, A, K_NEIGH, OBS_EMB)), beh_nb],
        axis=-1)
    trust = jax.nn.sigmoid(_mlp_apply(params["trust"], trust_in)[..., 0])
    ew = membership * trust
    ew = ew / (ew.sum(-1, keepdims=True) + EPS)

    feat = _mlp_apply(params["feat_proj"], jnp.concatenate([oe, beh], -1))
    pot = jax.nn.sigmoid(_mlp_apply(params["leader_pot"], feat)[..., 0])

    nb_pot = _gather(pot, neighbor_idx)
    leader_mask = jnp.all(pot[:, :, None] >= nb_pot, axis=-1)
    # Leader consensus in AGENT space (no argsort/compaction needed):
    # the slot<->agent map is order-preserving and every consumer below is an
    # order-invariant masked sum, so working on agent ids is exact.
    isl = leader_mask
    both = isl[:, :, None] & isl[:, None, :]
    ldist = jnp.where(both & ~eye, dist_raw, jnp.inf)
    lneg, lnb_idx = jax.lax.top_k(-ldist, LEADER_K)
    vmask = jnp.isfinite(lneg)
    m = jnp.max(jnp.where(vmask, lneg, -jnp.inf), axis=-1, keepdims=True)
    m = jnp.where(jnp.isfinite(m), m, 0.0)
    e = jnp.where(vmask, jnp.exp(lneg - m), 0.0)
    lw = e / (e.sum(-1, keepdims=True) + EPS)
    leader_neighbors = jnp.where(vmask, lnb_idx, -1)

    c = params["cons"]
    h = feat
    for _ in range(INTRA):
        nb = _gather(h, neighbor_idx)
        msg = (ew[..., None] * (nb @ c["msg_a"]["W"] + c["msg_a"]["b"])).sum(-2)
        h = jax.nn.gelu(
            jnp.concatenate([h, msg], -1) @ c["upd_a"]["W"] + c["upd_a"]["b"],
            approximate=False)
    islf = isl.astype(feat.dtype)
    hl = feat * islf[..., None]
    safe_ln = jnp.maximum(leader_neighbors, 0)
    for _ in range(INTER):
        lnb = _gather(hl, safe_ln)
        msg = (lw[..., None] * (lnb @ c["msg_l"]["W"] + c["msg_l"]["b"])).sum(-2)
        hl = jax.nn.gelu(
            jnp.concatenate([hl, msg], -1) @ c["upd_l"]["W"] + c["upd_l"]["b"],
            approximate=False) * islf[..., None]

    lmsg = _gather(hl, neighbor_idx)
    msk = _gather(islf, neighbor_idx)
    broadcast = (lmsg * msk[..., None] * ew[..., None]).sum(-2)
    fused = jnp.concatenate([h, broadcast, beh], -1)
    logits = _mlp_apply(params["actor"], fused)
    values = _mlp_apply(params["critic"], fused)[..., 0]
    return logits, values, ew, pot


_PMAPPED = None


def _get_pmapped():
    global _PMAPPED
    if _PMAPPED is None:
        _PMAPPED = jax.pmap(_forward_shard, axis_name="cores",
                            in_axes=(0, 0, 0, None))
    return _PMAPPED


def kernel(obs, positions, history, params):
    obs = np.asarray(obs, dtype=np.float32)
    positions = np.asarray(positions, dtype=np.float32)
    history = np.asarray(history, dtype=np.float32)
    params = jax.tree_util.tree_map(lambda x: np.asarray(x, np.float32), params)

    # shard batch across the 8 cores: [8, 4, A, ...]
    obs_s = obs.reshape(N_CORES, B_LOC, A, OBS_DIM)
    pos_s = positions.reshape(N_CORES, B_LOC, A, POS_DIM)
    hist_s = history.reshape(N_CORES, B_LOC, A, HIST_W, history.shape[-1])

    fn = _get_pmapped()
    logits, values, ew, pot = fn(obs_s, pos_s, hist_s, params)

    logits = np.asarray(logits).reshape(B, A, -1)
    values = np.asarray(values).reshape(B, A)
    ew = np.asarray(ew).reshape(B, A, K_NEIGH)
    pot = np.asarray(pot).reshape(B, A)
    return logits, values, ew, pot


if __name__ == "__main__":
    import sys
    sys.path.insert(0, "/root/problem")
    import reference
    cpu = jax.devices("cpu")[0]
    with jax.default_device(cpu):
        inputs = reference.setup_inputs()
        inputs = jax.tree_util.tree_map(
            lambda x: jax.device_put(np.asarray(x), cpu), inputs)
        exp = reference.reference(**inputs)
    act = kernel(**{k: np.asarray(v) if not isinstance(v, dict) else v
                    for k, v in inputs.items()})
    for i, (e, a) in enumerate(zip(exp, act)):
        e = np.asarray(e); a = np.asarray(a)
        err = np.abs(e - a).max() / (np.abs(e).max() + 1e-9)
        print(f"out[{i}] shape={a.shape} rel_err={err:.3e}")
